# revision 30
# baseline (speedup 1.0000x reference)
"""TRN2 Bass kernel for nn_IrrepsLinear (e3nn-style per-irrep linear layer).

Computation (per node n, N=200000 nodes, 480 features):
  out0 = (x0 @ W0 + modal_attr[batch[n]] @ W0m) / sqrt(130)   cols   0:128
  out1 = einsum('nim,io->nom', x1, W1) / sqrt(64)             cols 128:320
  out2 = einsum('nim,io->nom', x2, W2) / sqrt(32)             cols 320:480

Strategy: data-parallel over nodes across 8 NeuronCores (25000 nodes/core,
padded to 25088 = 196 blocks of 128 nodes). All layout work happens on the
host so the device runs a pure streaming GEMM at the HBM roofline:

  - the modal gather modal_attr[batch] is a host-side table lookup; its two
    values are appended to x as extra input features, with W0m/sqrt(130) as
    the matching weight rows (the FLOPs stay on device)
  - the 1e/2e irreps are de-interleaved m-major on the host, which turns the
    480x480 block weight into a block-diagonal matrix whose blocks are all
    <=128 wide: input chunks c0=x0, c1=(x1 m0,m1), c2=(x1 m2, x2 m0,m1),
    c3=(x2 m2,m3,m4 + modal, zero-padded to 128 rows)
  - x shards are cast to fp16 and stored TRANSPOSED per 128-node block
    ([feature, node] tiles) so the device needs no PE transposes: chunk
    tiles feed the PE directly as the stationary operand
  - per block: 4 matmuls, each a full 128-row stationary load (partial
    row-group loads stall the PE pipe), 608 moving columns total; outputs
    are computed in reordered columns [c1-out | c2-out | c3-out | c0-out]
    so chunk 3's rhs (x2 blocks + modal rows into out0) spans a contiguous
    224 columns; the host applies the inverse permutation
  - one PSUM tile [128 nodes, 480] per block, cast to fp16 by a copy
    alternating between DVE and ACT; outputs stored fp16, upcast on host
  - input DMAs ride the SWDGE queue (gpsimd), output stores the HWDGE
    queue (sync), both split in halves per 14-block super-tile so the
    queues stream continuously
"""
import numpy as np

import concourse.bass as bass
import concourse.mybir as mybir
import concourse.tile as tile
from concourse import bacc
from concourse.bass_utils import run_bass_kernel_spmd

f32 = mybir.dt.float32
f16 = mybir.dt.float16

N_CORES = 8
MUL0, MUL1, MUL2, NMOD = 128, 64, 32, 2
DTOT = 480
BLK = 128           # nodes per block
SUP = 14            # blocks per super-tile


def _in_perm():
    """std feature index for each de-interleaved (m-major) input column."""
    p1 = (MUL0 + 3 * np.arange(MUL1)[None, :] + np.arange(3)[:, None]).reshape(-1)
    p2 = (MUL0 + 3 * MUL1 + 5 * np.arange(MUL2)[None, :]
          + np.arange(5)[:, None]).reshape(-1)
    return np.concatenate([np.arange(MUL0), p1, p2])


def _out_perm():
    """std feature index for each device output column.

    Device column order: [c1-out (x1 m0,m1) | c2-out (x1 m2, x2 m0,m1) |
    c3-out (x2 m2,m3,m4) | c0-out (x0+modal)].
    """
    p = _in_perm()
    return np.concatenate([p[128:480], p[0:128]])


def _block_diag(*ms):
    n = sum(m.shape[0] for m in ms)
    out = np.zeros((n, n), dtype=ms[0].dtype)
    o = 0
    for m in ms:
        out[o:o + m.shape[0], o:o + m.shape[0]] = m
        o += m.shape[0]
    return out


def _host_prep(x, modal_attr, W0, W0m, W1, W2, batch):
    x = np.asarray(x)
    batch = np.asarray(batch)
    N = x.shape[0]
    ns = N // N_CORES
    nblk = (ns + BLK - 1) // BLK
    ns_pad = nblk * BLK

    inv0 = np.float32(1.0) / np.sqrt(np.float32(MUL0 + NMOD))
    inv1 = np.float32(1.0) / np.sqrt(np.float32(MUL1))
    inv2 = np.float32(1.0) / np.sqrt(np.float32(MUL2))
    W0s = (np.asarray(W0, np.float32) * inv0).astype(np.float16)
    W1s = (np.asarray(W1, np.float32) * inv1).astype(np.float16)
    W2s = (np.asarray(W2, np.float32) * inv2).astype(np.float16)
    r0 = np.ascontiguousarray(W0s)
    r1 = _block_diag(W1s, W1s)
    r2 = _block_diag(W1s, W2s, W2s)
    # chunk-3 rhs [128, 224]: x2 m2,m3,m4 -> cols 0:96, modal -> cols 96:224
    r3 = np.zeros((128, 224), dtype=np.float16)
    r3[0:96, 0:96] = _block_diag(W2s, W2s, W2s)
    r3[96:98, 96:224] = (np.asarray(W0m, np.float32) * inv0).astype(np.float16)

    p = _in_perm()
    modal = np.asarray(modal_attr, np.float32)[batch]  # [N, 2] host gather

    in_maps = []
    for i in range(N_CORES):
        xa = np.zeros((ns_pad, 482), dtype=np.float16)
        xa[:ns, :DTOT] = x[i * ns:(i + 1) * ns][:, p]
        xa[:ns, DTOT:] = modal[i * ns:(i + 1) * ns]
        # chunks 0-2 as [feature, node] tiles: A[p, (b*3+c)*128 + j]
        A = np.ascontiguousarray(
            xa[:, :384].reshape(nblk, BLK, 3, 128).transpose(3, 0, 2, 1)
        ).reshape(128, nblk * 384)
        # chunk 3 (96 x2 features + 2 modal): B[p, b*128 + j]
        B = np.ascontiguousarray(
            xa[:, 384:].reshape(nblk, BLK, 98).transpose(2, 0, 1)
        ).reshape(98, nblk * BLK)
        in_maps.append({"xsA": A, "xsB": B,
                        "r0": r0, "r1": r1, "r2": r2, "r3": r3})
    return in_maps, ns, ns_pad


def _build_nc(ns_pad):
    nblk = ns_pad // BLK
    # small first super (compute starts sooner) and small last super
    # (shorter drain tail); full-size supers in between
    sizes = []
    rem = nblk
    if rem > SUP:
        sizes.append(SUP // 2)
        rem -= SUP // 2
    while rem > SUP:
        sizes.append(SUP)
        rem -= SUP
    if rem > SUP // 2:
        sizes.append(rem - SUP // 2)
        rem = SUP // 2
    sizes.append(rem)
    supers = []
    b0 = 0
    for sz in sizes:
        supers.append((b0, sz))
        b0 += sz

    nc = bacc.Bacc("TRN2", target_bir_lowering=False, debug=False)
    xsA = nc.dram_tensor("xsA", [128, nblk * 384], f16, kind="ExternalInput").ap()
    xsB = nc.dram_tensor("xsB", [98, nblk * BLK], f16, kind="ExternalInput").ap()
    rdr = [nc.dram_tensor(n, list(s), f16, kind="ExternalInput").ap()
           for n, s in (("r0", (128, 128)), ("r1", (128, 128)),
                        ("r2", (128, 128)), ("r3", (128, 224)))]
    ys = nc.dram_tensor("ys", [128, nblk * DTOT], f16, kind="ExternalOutput").ap()

    with tile.TileContext(nc) as tc:
        with tc.tile_pool(name="const", bufs=1) as cpool, \
             tc.tile_pool(name="sba", bufs=8) as sba, \
             tc.tile_pool(name="sbb", bufs=8) as sbb, \
             tc.tile_pool(name="sbo", bufs=4) as sbo, \
             tc.tile_pool(name="ps", bufs=8, space="PSUM") as psp:

            wt = []
            for k, dr in enumerate(rdr):
                t = cpool.tile(list(dr.shape), f16, tag=f"w{k}")
                nc.scalar.dma_start(out=t[:], in_=dr)
                wt.append(t)
            w0, w1, w2, w3 = wt

            for b0, sblk in supers:
                xA = sba.tile([128, SUP * 384], f16, tag="xa")
                xB = sbb.tile([128, SUP * BLK], f16, tag="xb")
                # zero the K-padding rows once per super (32-aligned partition
                # base); the DMA then overwrites rows 0:98 with real data
                nc.gpsimd.memset(xB[96:128, :sblk * BLK], 0.0)
                h = (sblk + 1) // 2
                if b0 == 0:
                    # fine-grained first fill so compute starts asap
                    step = max(1, (sblk + 3) // 4)
                    in_pieces = [(i, min(i + step, sblk))
                                 for i in range(0, sblk, step)]
                else:
                    in_pieces = [(0, h), (h, sblk)]
                for lo, hi in in_pieces:
                    if hi <= lo:
                        continue
                    nc.gpsimd.dma_start(
                        out=xA[:, lo * 384:hi * 384],
                        in_=xsA[:, (b0 + lo) * 384:(b0 + hi) * 384])
                nc.sync.dma_start(
                    out=xB[0:98, :sblk * BLK],
                    in_=xsB[:, b0 * BLK:(b0 + sblk) * BLK])

                out_sb = sbo.tile([128, SUP * DTOT], f16, tag="out")

                for b in range(sblk):
                    ps_o = psp.tile([128, DTOT], f32, tag="po")
                    a0 = b * 384
                    nb = b * BLK
                    mm = nc.tensor.matmul
                    # c3 (+modal rows): fresh write of cols 256:480
                    mm(ps_o[:, 256:480], xB[:, nb:nb + BLK], w3[:],
                       start=True, stop=False, skip_group_check=True)
                    # c0 accumulates modal's out0 region, cols 352:480
                    mm(ps_o[:, 352:480], xA[:, a0:a0 + 128], w0[:],
                       start=False, stop=True, skip_group_check=True)
                    mm(ps_o[:, 0:128], xA[:, a0 + 128:a0 + 256], w1[:],
                       start=True, stop=True, skip_group_check=True)
                    mm(ps_o[:, 128:256], xA[:, a0 + 256:a0 + 384], w2[:],
                       start=True, stop=True, skip_group_check=True)

                    o_sl = out_sb[:, b * DTOT:(b + 1) * DTOT]
                    if b % 2 == 0:
                        nc.vector.tensor_copy(o_sl, ps_o[:])
                    else:
                        nc.scalar.copy(o_sl, ps_o[:])

                # output halves on the two HWDGE queues (long per-partition
                # runs keep the packet size, and thus queue throughput, high);
                # fine-grained pieces on the last super so the drain is short
                if b0 + sblk == nblk:
                    out_pieces = [(i, min(i + 2, sblk))
                                  for i in range(0, sblk, 2)]
                else:
                    out_pieces = [(0, h), (h, sblk)]
                for k, (lo, hi) in enumerate(out_pieces):
                    if hi <= lo:
                        continue
                    eng = nc.sync if k % 2 == 0 else nc.scalar
                    eng.dma_start(
                        out=ys[:, (b0 + lo) * DTOT:(b0 + hi) * DTOT],
                        in_=out_sb[:, lo * DTOT:hi * DTOT])

    nc.compile()
    return nc


_NC_CACHE = {}


def kernel(x, modal_attr, W0, W0m, W1, W2, batch):
    in_maps, ns, ns_pad = _host_prep(x, modal_attr, W0, W0m, W1, W2, batch)
    if ns_pad not in _NC_CACHE:
        _NC_CACHE[ns_pad] = _build_nc(ns_pad)
    nc = _NC_CACHE[ns_pad]
    res = run_bass_kernel_spmd(nc, in_maps, core_ids=list(range(N_CORES)))
    nblk = ns_pad // BLK
    pout = _out_perm()
    invp = np.empty(DTOT, dtype=np.int64)
    invp[pout] = np.arange(DTOT)
    outs = []
    for i in range(N_CORES):
        ysd = res.results[i]["ys"].reshape(128, nblk, DTOT)
        o = ysd[:, :, invp].transpose(1, 0, 2).reshape(ns_pad, DTOT)[:ns]
        outs.append(o.astype(np.float32))
    return np.ascontiguousarray(np.concatenate(outs, axis=0))
